# revision 4
# baseline (speedup 1.0000x reference)
"""Trainium2 Bass kernel for nn_ClusterDown (moe_routing).

Per-pixel: out[b, :, h, w] = relu(x @ W_l^T + b_l), l = cluster label of the
pixel; x = image[b, :, h, w] (C=128 channels), W: [5, 8, 128], b: [5, 8].

Strategy (data-parallel over 8 cores; core k handles batch k//2, H-half k%2):

Per 512-pixel tile, with image kept in its native [C, pixels] layout:
  1. mm1 (PE, f32r):  psum1[40, 512]  = W_allT.T @ x_tile          (all 5 classes)
  2. mmB (PE, f32r):  psum1          += (BIG*E).T @ onehot5_tile   (adds BIG to the
     8 rows of the labeled class only — accumulated into the same PSUM bank)
  3. ACT:             t[40, 512] = relu(psum1 + (b40 - BIG))       (bias per
     partition; unselected classes become relu(x@W+b-BIG) = 0 exactly)
  4. mm2 (PE, f32r):  psum3[8, 512] = S.T @ t                      (sums the 5
     groups of 8 rows; exactly one is nonzero -> pure selection)
  5. DVE copy psum3 -> SBUF, DMA out.

All elementwise work is one ACT pass over [40, N] and one DVE copy of [8, N];
the selection itself rides on the PE and PSUM accumulation. Matmuls run in
float32r (full PE speed at N=512, ~1e-4 relative precision).
"""

import os
from contextlib import ExitStack

import numpy as np

B, C, H, Wd = 4, 128, 256, 256
CLASSES, MS = 5, 8
NCOLS = CLASSES * MS  # 40
NCORES = 8
PX_PER_CORE = (B * H * Wd) // NCORES  # 32768
TILE = 512
NT = PX_PER_CORE // TILE  # 64
BIG = 512.0

# Module-level knobs for the dev harness (default: plain run, no tracing).
TRACE = False
TRACE_DIR = None
LAST_RESULTS = None

_NC = None


def _build():
    import concourse.tile as tile
    from concourse import bacc, mybir

    dt = mybir.dt
    f32 = dt.float32
    f32r = dt.float32r

    nc = bacc.Bacc("TRN2", target_bir_lowering=False, debug=False,
                   num_devices=NCORES)

    x_d = nc.dram_tensor("x", [C, PX_PER_CORE], f32r, kind="ExternalInput")
    oh_d = nc.dram_tensor("oh", [CLASSES, PX_PER_CORE], f32r, kind="ExternalInput")
    w_d = nc.dram_tensor("w", [C, NCOLS], f32r, kind="ExternalInput")
    bige_d = nc.dram_tensor("bige", [CLASSES, NCOLS], f32r, kind="ExternalInput")
    s_d = nc.dram_tensor("s", [NCOLS, MS], f32r, kind="ExternalInput")
    bias_d = nc.dram_tensor("bias", [NCOLS, 1], f32, kind="ExternalInput")
    out_d = nc.dram_tensor("out", [MS, PX_PER_CORE], f32, kind="ExternalOutput")

    with tile.TileContext(nc) as tc, ExitStack() as ctx:
        const = ctx.enter_context(tc.tile_pool(name="const", bufs=1))
        xpool = ctx.enter_context(tc.tile_pool(name="xp", bufs=8))
        tpool = ctx.enter_context(tc.tile_pool(name="tp", bufs=3))
        opool = ctx.enter_context(tc.tile_pool(name="op", bufs=4))
        ps1p = ctx.enter_context(tc.tile_pool(name="ps1", bufs=4, space="PSUM"))
        ps3p = ctx.enter_context(tc.tile_pool(name="ps3", bufs=4, space="PSUM"))

        w_sb = const.tile([C, NCOLS], f32r)
        bige_sb = const.tile([CLASSES, NCOLS], f32r)
        s_sb = const.tile([NCOLS, MS], f32r)
        bias_sb = const.tile([NCOLS, 1], f32)
        oh_sb = const.tile([CLASSES, PX_PER_CORE], f32r)
        nc.sync.dma_start(w_sb[:], w_d.ap())
        nc.sync.dma_start(bige_sb[:], bige_d.ap())
        nc.sync.dma_start(s_sb[:], s_d.ap())
        nc.sync.dma_start(bias_sb[:], bias_d.ap())
        nc.sync.dma_start(oh_sb[:], oh_d.ap())

        import concourse.bass as bass

        relu = mybir.ActivationFunctionType.Relu
        for i in range(NT):
            sl = bass.ts(i, TILE)
            x_t = xpool.tile([C, TILE], f32r)
            nc.sync.dma_start(x_t[:], x_d.ap()[:, sl])

            ps1 = ps1p.tile([NCOLS, TILE], f32)
            nc.tensor.matmul(ps1[:], w_sb[:], x_t[:], start=True, stop=False)
            nc.tensor.matmul(ps1[:], bige_sb[:], oh_sb[:, sl],
                             start=False, stop=True)

            t_t = tpool.tile([NCOLS, TILE], f32r)
            nc.scalar.activation(t_t[:], ps1[:], relu, bias=bias_sb[:])

            ps3 = ps3p.tile([MS, TILE], f32)
            nc.tensor.matmul(ps3[:], s_sb[:], t_t[:], start=True, stop=True)

            o_t = opool.tile([MS, TILE], f32)
            nc.vector.tensor_copy(o_t[:], ps3[:])
            nc.sync.dma_start(out_d.ap()[:, sl], o_t[:])

    nc.compile()
    return nc


def _host_consts(W, b):
    W = np.asarray(W, np.float32)
    b = np.asarray(b, np.float32)
    w_lhsT = np.ascontiguousarray(W.transpose(2, 0, 1).reshape(C, NCOLS))
    bige = np.kron(np.eye(CLASSES, dtype=np.float32),
                   np.ones((1, MS), np.float32)) * np.float32(BIG)
    smat = np.ascontiguousarray(np.tile(np.eye(MS, dtype=np.float32),
                                        (CLASSES, 1)))
    bias = (b.reshape(NCOLS, 1) - np.float32(BIG)).astype(np.float32)
    return w_lhsT, bige, smat, bias


def kernel(image, clusters, W, b):
    global _NC, LAST_RESULTS
    from concourse.bass_utils import run_bass_kernel_spmd

    if _NC is None:
        _NC = _build()

    image = np.asarray(image, np.float32)
    labels = np.asarray(clusters)[:, 0]  # [B, H, W] int
    oh_full = (labels[None, ...] ==
               np.arange(CLASSES, dtype=labels.dtype)[:, None, None, None]
               ).astype(np.float32)  # [5, B, H, W]
    w_lhsT, bige, smat, bias = _host_consts(W, b)

    in_maps = []
    for k in range(NCORES):
        b_idx, h_half = divmod(k, 2)
        h0 = h_half * (H // 2)
        x_np = np.ascontiguousarray(
            image[b_idx, :, h0:h0 + H // 2, :]).reshape(C, PX_PER_CORE)
        oh_np = np.ascontiguousarray(
            oh_full[:, b_idx, h0:h0 + H // 2, :]).reshape(CLASSES, PX_PER_CORE)
        in_maps.append({"x": x_np, "oh": oh_np, "w": w_lhsT, "bige": bige,
                        "s": smat, "bias": bias})

    kwargs = {}
    if TRACE:
        kwargs = {"trace": True, "tmpdir": TRACE_DIR}
    res = run_bass_kernel_spmd(_NC, in_maps, list(range(NCORES)), **kwargs)
    LAST_RESULTS = res

    out = np.empty((B, MS, H, Wd), np.float32)
    for k in range(NCORES):
        b_idx, h_half = divmod(k, 2)
        h0 = h_half * (H // 2)
        out[b_idx, :, h0:h0 + H // 2, :] = \
            res.results[k]["out"].reshape(MS, H // 2, Wd)
    return out


# revision 6
# speedup vs baseline: 1.6643x; 1.6643x over previous
"""Trainium2 Bass kernel for nn_ClusterDown (moe_routing).

Per-pixel: out[b, :, h, w] = relu(x @ W_l^T + b_l), l = cluster label of the
pixel; x = image[b, :, h, w] (C=128 channels), W: [5, 8, 128], b: [5, 8].

Data-parallel over 8 cores; core k handles batch k//2, H-half k%2
(32768 pixels/core). Image stays in its native [C, pixels] layout.

Per 512-pixel tile:
  1. mm1 (PE, f16):  psum1[40, 512]  = W_allT.T @ x_tile           (all 5 classes)
  2. mmB (PE, f16):  psum1          += (BIG*E).T @ onehot5_tile    (adds BIG=512
     to the 8 rows of the labeled class only — PSUM fp32 accumulation)
  3. ACT:            t[40, 512] = relu(psum1 + (b40 - BIG)) -> f16 (bias per
     partition; unselected classes give relu(x@W+b-BIG) = 0 exactly)
  4. mm2 (PE, f16):  psum3[8, 512] = S.T @ t                       (sums the 5
     groups of 8 rows; exactly one is nonzero -> pure selection)
  5. copy psum3 -> SBUF f32 (alternating DVE/ACT), DMA out.

The selection rides entirely on PE + PSUM accumulation; elementwise work is one
ACT pass over [40, N] and one [8, N] PSUM->SBUF copy. f16 matmuls run at full
PE speed (f32r measured 3x slower); masks/BIG are exact in f16, so the only
f16 losses are x/W rounding and the t rounding (~5e-4 rel-to-scale total).
"""

import os
from contextlib import ExitStack

import numpy as np

B, C, H, Wd = 4, 128, 256, 256
CLASSES, MS = 5, 8
NCOLS = CLASSES * MS  # 40
NCORES = 8
PX_PER_CORE = (B * H * Wd) // NCORES  # 32768
TILE = 512
NT = PX_PER_CORE // TILE  # 64
BIG = 512.0

# Dev-harness knobs (default: plain run, no tracing).
TRACE = False
TRACE_DIR = None
LAST_RESULTS = None

_NC = None


def _build():
    import concourse.bass as bass
    import concourse.tile as tile
    from concourse import bacc, mybir

    dt = mybir.dt
    f32 = dt.float32
    f16 = dt.float16

    nc = bacc.Bacc("TRN2", target_bir_lowering=False, debug=False,
                   num_devices=NCORES)

    x_d = nc.dram_tensor("x", [C, PX_PER_CORE], f16, kind="ExternalInput")
    oh_d = nc.dram_tensor("oh", [CLASSES, PX_PER_CORE], f16, kind="ExternalInput")
    w_d = nc.dram_tensor("w", [C, NCOLS], f16, kind="ExternalInput")
    bige_d = nc.dram_tensor("bige", [CLASSES, NCOLS], f16, kind="ExternalInput")
    s_d = nc.dram_tensor("s", [NCOLS, MS], f16, kind="ExternalInput")
    bias_d = nc.dram_tensor("bias", [NCOLS, 1], f32, kind="ExternalInput")
    out_d = nc.dram_tensor("out", [MS, PX_PER_CORE], f32, kind="ExternalOutput")

    with tile.TileContext(nc) as tc, ExitStack() as ctx:
        const = ctx.enter_context(tc.tile_pool(name="const", bufs=1))
        xpool = ctx.enter_context(tc.tile_pool(name="xp", bufs=12))
        tpool = ctx.enter_context(tc.tile_pool(name="tp", bufs=4))
        opool = ctx.enter_context(tc.tile_pool(name="op", bufs=3))
        ps1p = ctx.enter_context(tc.tile_pool(name="ps1", bufs=4, space="PSUM"))
        ps3p = ctx.enter_context(tc.tile_pool(name="ps3", bufs=4, space="PSUM"))

        w_sb = const.tile([C, NCOLS], f16)
        bige_sb = const.tile([CLASSES, NCOLS], f16)
        s_sb = const.tile([NCOLS, MS], f16)
        bias_sb = const.tile([NCOLS, 1], f32)
        oh_sb = const.tile([CLASSES, PX_PER_CORE], f16)
        nc.gpsimd.dma_start(w_sb[:], w_d.ap())
        nc.gpsimd.dma_start(bige_sb[:], bige_d.ap())
        nc.gpsimd.dma_start(s_sb[:], s_d.ap())
        nc.gpsimd.dma_start(bias_sb[:], bias_d.ap())
        # onehot preload in 8 chunks (8 KB per partition line) so no single
        # queue serializes the whole 320 KB.
        OHC = PX_PER_CORE // 8
        for j in range(8):
            nc.gpsimd.dma_start(oh_sb[:, bass.ts(j, OHC)],
                                oh_d.ap()[:, bass.ts(j, OHC)])

        relu = mybir.ActivationFunctionType.Relu
        x2 = None
        o2 = None
        for i in range(NT):
            sl = bass.ts(i, TILE)
            if i % 2 == 0:
                # double-tile x loads: [128, 1024] f16 = 2 KB/partition line
                x2 = xpool.tile([C, 2 * TILE], f16, tag="x2")
                src = x_d.ap()[:, i * TILE:(i + 2) * TILE]
                if i == 0:
                    # split the first load across two queues to cut the
                    # pipeline-start latency
                    nc.sync.dma_start(x2[0:64, :], src[0:64, :])
                    nc.scalar.dma_start(x2[64:128, :], src[64:128, :])
                else:
                    nc.sync.dma_start(x2[:], src)
            x_t = x2[:, (i % 2) * TILE:(i % 2 + 1) * TILE]

            ps1 = ps1p.tile([NCOLS, TILE], f32)
            nc.tensor.matmul(ps1[:], w_sb[:], x_t, start=True, stop=False)
            nc.tensor.matmul(ps1[:], bige_sb[:], oh_sb[:, sl],
                             start=False, stop=True)

            t_t = tpool.tile([NCOLS, TILE], f16)
            nc.scalar.activation(t_t[:], ps1[:], relu, bias=bias_sb[:])

            ps3 = ps3p.tile([MS, TILE], f32)
            nc.tensor.matmul(ps3[:], s_sb[:], t_t[:], start=True, stop=True)

            if i % 2 == 0:
                o2 = opool.tile([MS, 2 * TILE], f32, tag="o2")
            osl = o2[:, (i % 2) * TILE:(i % 2 + 1) * TILE]
            if i % 2 == 0:
                nc.vector.tensor_copy(osl, ps3[:])
            else:
                nc.scalar.copy(osl, ps3[:])
                nc.gpsimd.dma_start(
                    out_d.ap()[:, (i - 1) * TILE:(i + 1) * TILE], o2[:])

    nc.compile()
    return nc


def _host_consts(W, b):
    W = np.asarray(W, np.float32)
    b = np.asarray(b, np.float32)
    w_lhsT = np.ascontiguousarray(W.transpose(2, 0, 1).reshape(C, NCOLS)
                                  ).astype(np.float16)
    bige = (np.kron(np.eye(CLASSES, dtype=np.float32),
                    np.ones((1, MS), np.float32)) * np.float32(BIG)
            ).astype(np.float16)
    smat = np.ascontiguousarray(np.tile(np.eye(MS, dtype=np.float32),
                                        (CLASSES, 1))).astype(np.float16)
    bias = (b.reshape(NCOLS, 1) - np.float32(BIG)).astype(np.float32)
    return w_lhsT, bige, smat, bias


def kernel(image, clusters, W, b):
    global _NC, LAST_RESULTS
    from concourse.bass_utils import run_bass_kernel_spmd

    if _NC is None:
        _NC = _build()

    image = np.asarray(image, np.float32)
    labels = np.asarray(clusters)[:, 0]  # [B, H, W] int
    oh_full = (labels[None, ...] ==
               np.arange(CLASSES, dtype=labels.dtype)[:, None, None, None]
               ).astype(np.float16)  # [5, B, H, W]
    w_lhsT, bige, smat, bias = _host_consts(W, b)

    in_maps = []
    for k in range(NCORES):
        b_idx, h_half = divmod(k, 2)
        h0 = h_half * (H // 2)
        x_np = np.ascontiguousarray(
            image[b_idx, :, h0:h0 + H // 2, :].astype(np.float16)
        ).reshape(C, PX_PER_CORE)
        oh_np = np.ascontiguousarray(
            oh_full[:, b_idx, h0:h0 + H // 2, :]).reshape(CLASSES, PX_PER_CORE)
        in_maps.append({"x": x_np, "oh": oh_np, "w": w_lhsT, "bige": bige,
                        "s": smat, "bias": bias})

    kwargs = {}
    if TRACE:
        kwargs = {"trace": True, "tmpdir": TRACE_DIR}
    res = run_bass_kernel_spmd(_NC, in_maps, list(range(NCORES)), **kwargs)
    LAST_RESULTS = res

    out = np.empty((B, MS, H, Wd), np.float32)
    for k in range(NCORES):
        b_idx, h_half = divmod(k, 2)
        h0 = h_half * (H // 2)
        out[b_idx, :, h0:h0 + H // 2, :] = \
            res.results[k]["out"].reshape(MS, H // 2, Wd)
    return out


# revision 9
# speedup vs baseline: 2.1213x; 1.2746x over previous
"""Trainium2 Bass kernel for nn_ClusterDown (moe_routing).

Per-pixel: out[b, :, h, w] = relu(x @ W_l^T + b_l), l = cluster label of the
pixel; x = image[b, :, h, w] (C=128 channels), W: [5, 8, 128], b: [5, 8].

Data-parallel over 8 cores; core k handles batch k//2, H-half k%2
(32768 pixels/core). Image stays in its native [C, pixels] layout.

Math per 512-pixel tile (all matmuls f16, PSUM accumulates fp32):
  psum1[40, N] = W_allT.T @ x  +  (BIG*E).T @ onehot5     (BIG added to the
                                                           labeled class rows)
  t[40, N]     = relu(psum1 + (b40 - BIG))                (ScalarE; unselected
                                                           classes -> exactly 0)
  psum3[8, N]  = S.T @ t                                  (group-sum = select)
  out          = copy psum3 -> SBUF (DVE), DMA.

PE array packing (the PE here runs at a fixed 1.2 GHz — HAM never engages):
tiles are processed in PAIRS via column tiling (tile A -> array cols 0..39,
tile B -> cols 64..103, separate PSUM banks), and the mm2 reduction is packed
4 tiles per pass across (row, col) sub-array quadrants. This gives
~0.5 + 0.5 + 0.25 PE passes per tile instead of 3.
"""

import os
from contextlib import ExitStack

import numpy as np

B, C, H, Wd = 4, 128, 256, 256
CLASSES, MS = 5, 8
NCOLS = CLASSES * MS  # 40
NCORES = 8
PX_PER_CORE = (B * H * Wd) // NCORES  # 32768
TILE = 512
NT = PX_PER_CORE // TILE  # 64
BIG = 512.0

# Dev-harness knobs (default: plain run, no tracing).
TRACE = False
TRACE_DIR = None
LAST_RESULTS = None

_NC = None


def _build():
    import concourse.bass as bass
    import concourse.tile as tile
    from concourse import bacc, mybir

    dt = mybir.dt
    f32 = dt.float32
    f16 = dt.float16

    nc = bacc.Bacc("TRN2", target_bir_lowering=False, debug=False,
                   num_devices=NCORES)

    x_d = nc.dram_tensor("x", [C, PX_PER_CORE], f16, kind="ExternalInput")
    oh_d = nc.dram_tensor("oh", [CLASSES, PX_PER_CORE], f16, kind="ExternalInput")
    w_d = nc.dram_tensor("w", [C, NCOLS], f16, kind="ExternalInput")
    bige_d = nc.dram_tensor("bige", [CLASSES, NCOLS], f16, kind="ExternalInput")
    s_d = nc.dram_tensor("s", [104, 32], f16, kind="ExternalInput")
    bias_d = nc.dram_tensor("bias", [104, 1], f32, kind="ExternalInput")
    out_d = nc.dram_tensor("out", [MS, PX_PER_CORE], f32, kind="ExternalOutput")

    with tile.TileContext(nc) as tc, ExitStack() as ctx:
        const = ctx.enter_context(tc.tile_pool(name="const", bufs=1))
        xpool = ctx.enter_context(tc.tile_pool(name="xp", bufs=12))
        tpool = ctx.enter_context(tc.tile_pool(name="tp", bufs=4))
        opool = ctx.enter_context(tc.tile_pool(name="op", bufs=3))
        ps1p = ctx.enter_context(tc.tile_pool(name="ps1", bufs=3, space="PSUM"))
        ps3p = ctx.enter_context(tc.tile_pool(name="ps3", bufs=2, space="PSUM"))

        w_sb = const.tile([C, NCOLS], f16)
        bige_sb = const.tile([CLASSES, NCOLS], f16)
        s_sb = const.tile([104, 32], f16)      # S at rows 0:40 and 64:104
        bias_sb = const.tile([104, 1], f32)    # b40-BIG at rows 0:40, 64:104
        oh_sb = const.tile([CLASSES, PX_PER_CORE], f16)
        nc.gpsimd.dma_start(w_sb[:], w_d.ap())
        nc.gpsimd.dma_start(bige_sb[:], bige_d.ap())
        nc.gpsimd.dma_start(s_sb[:], s_d.ap())
        nc.gpsimd.dma_start(bias_sb[:], bias_d.ap())
        OHC = PX_PER_CORE // 8
        for j in range(8):
            nc.gpsimd.dma_start(oh_sb[:, bass.ts(j, OHC)],
                                oh_d.ap()[:, bass.ts(j, OHC)])

        relu = mybir.ActivationFunctionType.Relu
        NP = NT // 2   # pairs
        t_pairs = {}
        for g in range(NT // 4):           # groups of 4 tiles (2 pairs)
            for pp in range(2):
                p = 2 * g + pp             # pair index; tiles 2p, 2p+1
                i0 = 2 * p
                xp_t = xpool.tile([C, 2 * TILE], f16, tag="x2")
                src = x_d.ap()[:, i0 * TILE:(i0 + 2) * TILE]
                if p < 2:
                    # split early loads across queues to cut pipeline-start
                    # latency
                    nc.sync.dma_start(xp_t[0:32, :], src[0:32, :])
                    nc.scalar.dma_start(xp_t[32:64, :], src[32:64, :])
                    nc.sync.dma_start(xp_t[64:96, :], src[64:96, :])
                    nc.scalar.dma_start(xp_t[96:128, :], src[96:128, :])
                else:
                    nc.sync.dma_start(xp_t[:], src)

                ps1a = ps1p.tile([NCOLS, TILE], f32, tag="ps1a")
                ps1b = ps1p.tile([104, TILE], f32, tag="ps1b")
                sl_a = bass.ts(i0, TILE)
                sl_b = bass.ts(i0 + 1, TILE)
                nc.tensor.matmul(ps1a[:], w_sb[:], xp_t[:, 0:TILE],
                                 start=True, stop=False, tile_position=(0, 0))
                nc.tensor.matmul(ps1b[64:104], w_sb[:], xp_t[:, TILE:2 * TILE],
                                 start=True, stop=False, tile_position=(0, 64))
                nc.tensor.matmul(ps1a[:], bige_sb[:], oh_sb[:, sl_a],
                                 start=False, stop=True, tile_position=(0, 0))
                nc.tensor.matmul(ps1b[64:104], bige_sb[:], oh_sb[:, sl_b],
                                 start=False, stop=True, tile_position=(0, 64))

                t_t = tpool.tile([104, TILE], f16, tag="t")
                nc.scalar.activation(t_t[0:NCOLS], ps1a[:], relu,
                                     bias=bias_sb[0:NCOLS])
                nc.scalar.activation(t_t[64:104], ps1b[64:104], relu,
                                     bias=bias_sb[64:104])
                t_pairs[pp] = t_t

            # mm2: 4 tiles packed across (row, col) sub-array quadrants into
            # one PSUM bank (single-shot writes to disjoint partition slices)
            ps3 = ps3p.tile([128, TILE], f32, tag="ps3")
            t0, t1 = t_pairs[0], t_pairs[1]
            nc.tensor.matmul(ps3[0:32], s_sb[0:NCOLS], t0[0:NCOLS],
                             start=True, stop=True, tile_position=(0, 0))
            nc.tensor.matmul(ps3[32:64], s_sb[64:104], t0[64:104],
                             start=True, stop=True, tile_position=(64, 32))
            nc.tensor.matmul(ps3[64:96], s_sb[0:NCOLS], t1[0:NCOLS],
                             start=True, stop=True, tile_position=(0, 64))
            nc.tensor.matmul(ps3[96:128], s_sb[64:104], t1[64:104],
                             start=True, stop=True, tile_position=(64, 96))

            o4 = opool.tile([128, TILE], f32, tag="o4")
            nc.vector.tensor_copy(o4[:], ps3[:])

            # out DMAs: plain [8, 512] partition slices (q-th quadrant holds
            # tile 4g+q at partitions q*32 .. q*32+8)
            for q in range(4):
                eng = nc.gpsimd if q < 2 else nc.sync
                eng.dma_start(out_d.ap()[:, bass.ts(4 * g + q, TILE)],
                              o4[q * 32:q * 32 + MS])

    nc.compile()
    return nc


def _host_consts(W, b):
    W = np.asarray(W, np.float32)
    b = np.asarray(b, np.float32)
    w_lhsT = np.ascontiguousarray(W.transpose(2, 0, 1).reshape(C, NCOLS)
                                  ).astype(np.float16)
    bige = (np.kron(np.eye(CLASSES, dtype=np.float32),
                    np.ones((1, MS), np.float32)) * np.float32(BIG)
            ).astype(np.float16)
    s40 = np.zeros((NCOLS, 32), np.float32)
    s40[:, 0:MS] = np.tile(np.eye(MS, dtype=np.float32), (CLASSES, 1))
    smat = np.zeros((104, 32), np.float32)
    smat[0:NCOLS] = s40
    smat[64:104] = s40
    bias = np.zeros((104, 1), np.float32)
    bias[0:NCOLS, 0] = b.reshape(NCOLS) - np.float32(BIG)
    bias[64:104, 0] = b.reshape(NCOLS) - np.float32(BIG)
    return w_lhsT, bige, smat.astype(np.float16), bias


def kernel(image, clusters, W, b):
    global _NC, LAST_RESULTS
    from concourse.bass_utils import run_bass_kernel_spmd

    if _NC is None:
        _NC = _build()

    image = np.asarray(image, np.float32)
    labels = np.asarray(clusters)[:, 0]  # [B, H, W] int
    oh_full = (labels[None, ...] ==
               np.arange(CLASSES, dtype=labels.dtype)[:, None, None, None]
               ).astype(np.float16)  # [5, B, H, W]
    w_lhsT, bige, smat, bias = _host_consts(W, b)

    in_maps = []
    for k in range(NCORES):
        b_idx, h_half = divmod(k, 2)
        h0 = h_half * (H // 2)
        x_np = np.ascontiguousarray(
            image[b_idx, :, h0:h0 + H // 2, :].astype(np.float16)
        ).reshape(C, PX_PER_CORE)
        oh_np = np.ascontiguousarray(
            oh_full[:, b_idx, h0:h0 + H // 2, :]).reshape(CLASSES, PX_PER_CORE)
        in_maps.append({"x": x_np, "oh": oh_np, "w": w_lhsT, "bige": bige,
                        "s": smat, "bias": bias})

    kwargs = {}
    if TRACE:
        kwargs = {"trace": True, "tmpdir": TRACE_DIR}
    res = run_bass_kernel_spmd(_NC, in_maps, list(range(NCORES)), **kwargs)
    LAST_RESULTS = res

    out = np.empty((B, MS, H, Wd), np.float32)
    for k in range(NCORES):
        b_idx, h_half = divmod(k, 2)
        h0 = h_half * (H // 2)
        out[b_idx, :, h0:h0 + H // 2, :] = \
            res.results[k]["out"].reshape(MS, H // 2, Wd)
    return out
